# revision 1
# baseline (speedup 1.0000x reference)
"""Self-contained Bass/Trainium2 kernel for the 2-layer LSTM + linear head.

Problem: x [2048, 512, 8] -> 2-layer LSTM (H=50, PyTorch gate order i,f,g,o)
-> last hidden state of layer 2 -> linear [1, 50] -> y [2048, 1].

Strategy: pure data parallel over 8 NeuronCores (256 batch rows each). On
each core the batch is further split into two independent 128-wide
sub-batch pipelines so the serial T=512 recurrence latency is overlapped.

Per-core layout (per sub-batch sb, 128 batch columns on the free dim):
  - Gate rows on partitions, padded to 64-row blocks so every engine access
    starts at a 32-aligned partition base: chunk A = [i rows 0:50 | f rows
    64:114], chunk B = [g rows 0:50 | o rows 64:114]. The g block's weights
    are pre-scaled by 2 so tanh(z) = 2*sigmoid(2z) - 1 needs only sigmoid.
  - One rhs tile R [128, 128] per step: rows 0:50 h0, 50:58 x_t, 58 ones
    (bias row), 59:64 zeros, 64:114 h1 (layer 2 runs one step behind layer
    1 - the skew makes both layers' gate matmuls computable in the same
    iteration). Biases are folded into the matmul via the ones row.
  - 4 matmuls per sb per step into one PSUM tile g [128, 512] (cols
    A-L0 | A-L1 | B-L0 | B-L1), one sigmoid over all gates, then the cell
    update on VectorE: u' = 2*i*sig2g (fused scalar_tensor_tensor),
    t1 = u' - i  (so t1 = i*tanh(zg)), v = f*c, c' = t1 + v, tanh(c') on
    ScalarE, h0/h1 products written straight into the next step's rhs tile.
"""
import numpy as np
import concourse.bacc as bacc
import concourse.mybir as mybir
from concourse.tile import TileContext
from concourse.bass_utils import run_bass_kernel_spmd

f32 = mybir.dt.float32
AF = mybir.ActivationFunctionType
ALU = mybir.AluOpType

H = 50
D = 8
B = 2048
T = 512
NCORES = 8
BC = B // NCORES   # 256 batch rows per core
NSB = 2
SB = BC // NSB     # 128 batch cols per sub-batch

_NC_CACHE = {}


def _build_nc():
    nc = bacc.Bacc(None, target_bir_lowering=False)

    xT = nc.dram_tensor("xT", [T, 14, BC], f32, kind="ExternalInput")
    w0a = nc.dram_tensor("w0a", [59, 128], f32, kind="ExternalInput")
    w0b = nc.dram_tensor("w0b", [59, 128], f32, kind="ExternalInput")
    w1a = nc.dram_tensor("w1a", [115, 128], f32, kind="ExternalInput")
    w1b = nc.dram_tensor("w1b", [115, 128], f32, kind="ExternalInput")
    wfin = nc.dram_tensor("wfin", [128, 1], f32, kind="ExternalInput")
    y = nc.dram_tensor("y", [1, BC], f32, kind="ExternalOutput")

    with TileContext(nc) as tc:
        with (
            tc.tile_pool(name="wp", bufs=1) as wp,
            tc.tile_pool(name="st", bufs=1) as st,
            tc.tile_pool(name="rp", bufs=3) as rp,
            tc.tile_pool(name="sp", bufs=2) as sp,
            tc.tile_pool(name="tp", bufs=2) as tp,
            tc.tile_pool(name="gp", bufs=2, space="PSUM") as gp,
        ):
            W0A = wp.tile([59, 128], f32, name="W0A")
            W0B = wp.tile([59, 128], f32, name="W0B")
            W1A = wp.tile([115, 128], f32, name="W1A")
            W1B = wp.tile([115, 128], f32, name="W1B")
            WF = wp.tile([128, 1], f32, name="WF")
            nc.sync.dma_start(out=W0A, in_=w0a[:, :])
            nc.sync.dma_start(out=W0B, in_=w0b[:, :])
            nc.sync.dma_start(out=W1A, in_=w1a[:, :])
            nc.sync.dma_start(out=W1B, in_=w1b[:, :])
            nc.sync.dma_start(out=WF, in_=wfin[:, :])

            C = [st.tile([128, 256], f32, name=f"C{sb}") for sb in range(NSB)]
            TH = [st.tile([128, 256], f32, name=f"TH{sb}") for sb in range(NSB)]
            for sb in range(NSB):
                nc.vector.memset(C[sb], 0.0)

            def new_r(sb, t, memset):
                r = rp.tile([128, SB], f32, name=f"rt{sb}", tag=f"r_{sb}")
                if memset:
                    nc.vector.memset(r, 0.0)
                nc.sync.dma_start(out=r[50:64, :],
                                  in_=xT[min(t, T - 1)][:, sb * SB:(sb + 1) * SB])
                return r

            rcur = [new_r(sb, 0, True) for sb in range(NSB)]

            for t in range(T + 1):
                rnext = [new_r(sb, t + 1, t + 1 <= 2) for sb in range(NSB)]
                g = [gp.tile([128, 512], f32, name=f"g{sb}", tag=f"g{sb}")
                     for sb in range(NSB)]
                for sb in range(NSB):
                    nc.tensor.matmul(g[sb][:, 0:128], W0A[0:59, :],
                                     rcur[sb][0:59, :], start=True, stop=True)
                for sb in range(NSB):
                    nc.tensor.matmul(g[sb][:, 256:384], W0B[0:59, :],
                                     rcur[sb][0:59, :], start=True, stop=True)
                for sb in range(NSB):
                    nc.tensor.matmul(g[sb][:, 128:256], W1A[0:114, :],
                                     rcur[sb][0:114, :], start=True, stop=True)
                for sb in range(NSB):
                    nc.tensor.matmul(g[sb][:, 384:512], W1B[0:114, :],
                                     rcur[sb][0:114, :], start=True, stop=True)

                for sb in range(NSB):
                    s = sp.tile([128, 512], f32, name=f"s{sb}", tag=f"s{sb}")
                    nc.scalar.activation(out=s, in_=g[sb][:, :], func=AF.Sigmoid)

                    up = tp.tile([128, 256], f32, name=f"up{sb}", tag=f"up{sb}")
                    t1 = tp.tile([128, 256], f32, name=f"t1{sb}", tag=f"t1{sb}")
                    v = tp.tile([128, 256], f32, name=f"v{sb}", tag=f"v{sb}")
                    # u' = (sig_2g * 2) * i
                    nc.vector.scalar_tensor_tensor(out=up[0:64, :],
                                                   in0=s[0:64, 256:512],
                                                   scalar=2.0, in1=s[0:64, 0:256],
                                                   op0=ALU.mult, op1=ALU.mult)
                    # t1 = u' - i = i * tanh(zg)
                    nc.vector.tensor_tensor(out=t1[64:128, :], in0=up[0:64, :],
                                            in1=s[0:64, 0:256], op=ALU.subtract)
                    # v = f * c
                    nc.vector.tensor_tensor(out=v[64:128, :], in0=s[64:128, 0:256],
                                            in1=C[sb][64:128, :], op=ALU.mult)
                    # c' = t1 + v
                    nc.vector.tensor_tensor(out=C[sb][64:128, :], in0=t1[64:128, :],
                                            in1=v[64:128, :], op=ALU.add)
                    # th = tanh(c')
                    nc.scalar.activation(out=TH[sb][64:128, :], in_=C[sb][64:128, :],
                                         func=AF.Tanh)
                    # h = o * th; layer-1 half feeds rows 0:50, layer-2 rows 64:114
                    nc.vector.tensor_tensor(out=rnext[sb][0:50, :],
                                            in0=s[64:114, 256:384],
                                            in1=TH[sb][64:114, 0:128], op=ALU.mult)
                    nc.vector.tensor_tensor(out=rnext[sb][64:114, :],
                                            in0=s[64:114, 384:512],
                                            in1=TH[sb][64:114, 128:256], op=ALU.mult)

                if t == 0:
                    # layer 2 ran on junk at t=0 (its real step 0 happens at t=1)
                    for sb in range(NSB):
                        nc.vector.memset(C[sb][64:128, 128:256], 0.0)
                        nc.vector.memset(rnext[sb][64:114, :], 0.0)
                rcur = rnext

            ysb = st.tile([1, BC], f32, name="ysb")
            for sb in range(NSB):
                fin = gp.tile([1, SB], f32, name=f"fin{sb}", tag=f"g{sb}")
                nc.tensor.matmul(fin[:, :], WF[64:114, :], rcur[sb][64:114, :],
                                 start=True, stop=True)
                nc.scalar.copy(out=ysb[:, sb * SB:(sb + 1) * SB], in_=fin[:, :])
            nc.sync.dma_start(out=y[:, :], in_=ysb)

    nc.compile()
    return nc


def _prep_weights(Wih0, Whh0, bih0, bhh0, Wih1, Whh1, bih1, bhh1):
    """Stacked/padded lhsT blobs; biases in K-row 58 (the rhs ones row)."""
    b0 = (np.asarray(bih0) + np.asarray(bhh0)).astype(np.float32)
    b1 = (np.asarray(bih1) + np.asarray(bhh1)).astype(np.float32)

    def chunk(hrows, xrows, onerow, Wh, Wx, b, g0, g1, krows, sc0=1.0, sc1=1.0):
        out = np.zeros((krows, 128), dtype=np.float32)
        for col0, gi, sc in ((0, g0, sc0), (64, g1, sc1)):
            rows = slice(gi * H, (gi + 1) * H)
            out[hrows, col0:col0 + H] = np.asarray(Wh)[rows, :].T * sc
            out[xrows, col0:col0 + H] = np.asarray(Wx)[rows, :].T * sc
            out[onerow, col0:col0 + H] = b[rows] * sc
        return out

    w0a = chunk(slice(0, 50), slice(50, 58), 58, Whh0, Wih0, b0, 0, 1, 59)
    w0b = chunk(slice(0, 50), slice(50, 58), 58, Whh0, Wih0, b0, 2, 3, 59, 2.0, 1.0)
    w1a = chunk(slice(64, 114), slice(0, 50), 58, Whh1, Wih1, b1, 0, 1, 115)
    w1b = chunk(slice(64, 114), slice(0, 50), 58, Whh1, Wih1, b1, 2, 3, 115, 2.0, 1.0)
    return w0a, w0b, w1a, w1b


def kernel(x, Wih0, Whh0, bih0, bhh0, Wih1, Whh1, bih1, bhh1, Wlin, blin):
    x = np.asarray(x, dtype=np.float32)
    w0a, w0b, w1a, w1b = _prep_weights(Wih0, Whh0, bih0, bhh0,
                                       Wih1, Whh1, bih1, bhh1)
    wfin = np.zeros((128, 1), np.float32)
    wfin[64:114, 0] = np.asarray(Wlin, dtype=np.float32)[0, :]

    if "nc" not in _NC_CACHE:
        _NC_CACHE["nc"] = _build_nc()
    nc = _NC_CACHE["nc"]

    in_maps = []
    for c in range(NCORES):
        xc = x[c * BC:(c + 1) * BC]              # [BC, T, D]
        xt = np.zeros((T, 14, BC), dtype=np.float32)
        xt[:, 0:D, :] = xc.transpose(1, 2, 0)
        xt[:, D, :] = 1.0                        # ones row (bias)
        in_maps.append({"xT": xt, "w0a": w0a, "w0b": w0b, "w1a": w1a,
                        "w1b": w1b, "wfin": wfin})

    res = run_bass_kernel_spmd(nc, in_maps, core_ids=list(range(NCORES)))
    out = np.empty((B, 1), dtype=np.float32)
    blin_v = np.float32(np.asarray(blin).reshape(-1)[0])
    for c in range(NCORES):
        out[c * BC:(c + 1) * BC, 0] = res.results[c]["y"][0] + blin_v
    return out
